# revision 4
# baseline (speedup 1.0000x reference)
"""Trainium2 Bass kernel for nn_DifferentiableCDF (soft Gaussian histogram -> CDF).

Algorithm (per core, data-parallel over pixels):
  u = 255*x in bin units; hi = floor(u/16) (16-bin block), d = u/16 - hi in [0,1].
  Gaussian weight for bin j = 16*(hi-1) + w (w in [W_LO, W_HI)) is
    exp(-ALPHAP*(d + c_w)^2),  c_w = (16-w)/16,  ALPHAP = 256/(255*sigma)^2.
  Columns split between DVE (scalar_tensor_tensor computing d^2+2cd, host
  descales by exp(-ALPHAP*c^2)) and ACT (Square activation computing (d+c)^2),
  one bulk Exp pass, then per-128-pixel-chunk matmuls with a 16-wide one-hot
  of hi scatter the weights into per-(unit, hi-block) PSUM accumulators.
  Host folds the 16x40 block tables into the 256-bin histogram, normalizes,
  and takes the cumulative sum.
"""
import sys
if "/opt/trn_rl_repo" not in sys.path:
    sys.path.insert(0, "/opt/trn_rl_repo")

import numpy as np
from concourse import bacc, tile
from concourse.bass_utils import run_bass_kernel_spmd
import concourse.mybir as mybir

# ---- problem constants (hardcoded per spec) ----
B, C, H, W = 4, 3, 256, 256
UNITS = B * C                  # 12 independent histograms
NPIX = H * W                   # 65536 pixels per unit
NCORES = 8
PIX_PER_CORE = NPIX // NCORES  # 8192 pixels per unit per core
CHUNKS_PER_UNIT = PIX_PER_CORE // 128  # 64
NCHUNK = UNITS * CHUNKS_PER_UNIT       # 768 chunks of 128 pixels
SIGMA = 0.01
BINS = 256
ALPHAP = 256.0 / (255.0 * SIGMA) ** 2  # 39.369...
W_LO, W_HI = 5, 45
NW = W_HI - W_LO                        # 40 columns
ACT_W_LO = 31                           # cols >= this use ACT Square path
DT = mybir.dt

_COMPILED = None  # cached (nc, meta)


def _build():
    nc = bacc.Bacc("TRN2", target_bir_lowering=False, debug=False,
                   num_devices=NCORES)
    x_ext = nc.declare_dram_parameter("xc", [128, NCHUNK], DT.float32,
                                      isOutput=False)
    tbl_ext = nc.declare_dram_parameter("table", [16, UNITS * NW], DT.float32,
                                        isOutput=True)

    with tile.TileContext(nc) as tc:
        with (
            tc.tile_pool(name="pool", bufs=1) as pool,
            tc.tile_pool(name="psum", bufs=1, space="PSUM") as psum_pool,
        ):
            xc = pool.tile([128, NCHUNK], DT.float32)
            nc.sync.dma_start(xc[:], x_ext[:])

            hi_i = pool.tile([128, NCHUNK], DT.int32)
            hi_f = pool.tile([128, NCHUNK], DT.float32)
            d = pool.tile([128, NCHUNK], DT.float32)

            # hi = floor(x*15.9375) via RNE(x*15.9375 - 0.5) [HW converts RNE]
            nc.vector.tensor_scalar(hi_i[:], xc[:], 15.9375, -0.5,
                                    mybir.AluOpType.mult, mybir.AluOpType.add)
            nc.vector.tensor_copy(hi_f[:], hi_i[:])
            # d = x*15.9375 - hi  in [0, 1]
            nc.vector.scalar_tensor_tensor(d[:], xc[:], 15.9375, hi_f[:],
                                           mybir.AluOpType.mult,
                                           mybir.AluOpType.subtract)

            # one-hot of hi, m-major layout [128, 16, NCHUNK] bf16
            oh = pool.tile([128, 16, NCHUNK], DT.float32)
            for m in range(16):
                nc.vector.tensor_scalar(oh[:, m, :], hi_f[:], float(m), None,
                                        mybir.AluOpType.is_equal)

            # per-column bias tiles for the ACT Square path
            bias_tiles = {}
            for w in range(ACT_W_LO, W_HI):
                bt = pool.tile([128, 1], DT.float32, tag=f"bias{w}")
                nc.gpsimd.memset(bt[:], (16.0 - w) / 16.0)
                bias_tiles[w] = bt

            # ARG [128, NCHUNK, NW] fp32
            arg = pool.tile([128, NCHUNK, NW], DT.float32)
            for w in range(W_LO, W_HI):
                wi = w - W_LO
                cw = (16.0 - w) / 16.0
                if w >= ACT_W_LO:
                    nc.scalar.activation(arg[:, :, wi], d[:],
                                         mybir.ActivationFunctionType.Square,
                                         bias=bias_tiles[w][:], scale=1.0)
                else:
                    nc.vector.scalar_tensor_tensor(arg[:, :, wi], d[:],
                                                   2.0 * cw, d[:],
                                                   mybir.AluOpType.add,
                                                   mybir.AluOpType.mult)
            # bulk exp in place: exp(-ALPHAP * arg)
            nc.scalar.activation(arg[:], arg[:],
                                 mybir.ActivationFunctionType.Exp,
                                 scale=-ALPHAP)

            acc = psum_pool.tile([16, UNITS * NW], DT.float32)
            for t in range(UNITS):
                for j in range(CHUNKS_PER_UNIT):
                    c = t * CHUNKS_PER_UNIT + j
                    nc.tensor.matmul(acc[:, t * NW:(t + 1) * NW],
                                     oh[:, :, c], arg[:, c, :],
                                     start=(j == 0), stop=(j == CHUNKS_PER_UNIT - 1))

            out_sb = pool.tile([16, UNITS * NW], DT.float32)
            nc.vector.tensor_copy(out_sb[:], acc[:])
            nc.sync.dma_start(tbl_ext[:], out_sb[:])

    nc.compile()
    return nc


def _get_compiled():
    global _COMPILED
    if _COMPILED is None:
        _COMPILED = _build()
    return _COMPILED


def _shard_x(x):
    """x (B,C,H,W) -> per-core [128, NCHUNK] arrays; element [p, 64t+j] =
    unit t, pixel 8192*core + 128*j + p."""
    xu = np.ascontiguousarray(x.reshape(UNITS, NPIX))
    shards = []
    for core in range(NCORES):
        sl = xu[:, core * PIX_PER_CORE:(core + 1) * PIX_PER_CORE]
        # (UNITS, 64, 128) -> (128, UNITS, 64)
        sl = sl.reshape(UNITS, CHUNKS_PER_UNIT, 128).transpose(2, 0, 1)
        shards.append(np.ascontiguousarray(sl.reshape(128, NCHUNK), np.float32))
    return shards


def _postprocess(tables):
    """tables: list of NCORES arrays [16, UNITS*NW] -> cdf (B, C, BINS) fp32."""
    tab = np.zeros((16, UNITS, NW), np.float64)
    for t in tables:
        tab += t.reshape(16, UNITS, NW).astype(np.float64)
    # descale DVE-path columns by exp(-ALPHAP*c^2)
    ws = np.arange(W_LO, W_HI)
    cw = (16.0 - ws) / 16.0
    beta = np.where(ws < ACT_W_LO, np.exp(-ALPHAP * cw ** 2), 1.0)
    tab *= beta[None, None, :]
    # fold: bin j = 16*(J-1) + w
    hist = np.zeros((UNITS, 16 + BINS + 48), np.float64)
    for J in range(16):
        hist[:, 16 * J + W_LO: 16 * J + W_HI] += tab[J, :, :]
    hist = hist[:, 16:16 + BINS]
    pdf = hist / (hist.sum(-1, keepdims=True) + 1e-6)
    cdf = np.cumsum(pdf, -1)
    return cdf.reshape(B, C, BINS).astype(np.float32)


def run_device(x, trace=False):
    nc = _get_compiled()
    in_maps = [{"xc": s} for s in _shard_x(np.asarray(x))]
    res = run_bass_kernel_spmd(nc, in_maps, list(range(NCORES)), trace=trace)
    tables = [res.results[i]["table"] for i in range(NCORES)]
    return tables, res


def kernel(x, centers):
    # centers is linspace(0,1,256) by construction; bin geometry is hardcoded.
    tables, _ = run_device(x)
    return _postprocess(tables)


if __name__ == "__main__":
    import jax, jax.numpy as jnp
    key = jax.random.key(0)
    k1, _ = jax.random.split(key)
    x = np.asarray(jax.random.uniform(k1, (B, C, H, W), dtype=jnp.float32))
    centers = np.linspace(0, 1, BINS, dtype=np.float32)
    out = kernel(x, centers)
    print("kernel output", out.shape, out.dtype, out[0, 0, :5], out[0, 0, -1])
